# revision 35
# baseline (speedup 1.0000x reference)
"""KNN top-16 kernel for trn2 (8 NeuronCores, SPMD), v3: packed single-scan.

Strategy: shard the 4x4096 query rows 8 ways (each core: one batch's half,
2048 rows); replicate that batch's 16384-point support set on the core.
Distances via a single augmented fp32 matmul per (128-query, 512-support)
tile: negdist2 = qaugT @ saug with qaug=[-q2,-1,2qx,2qy,2qz],
saug=[1,s2,sx,sy,sz], 4-way row-packed in the PE array (K=5 per 32-row
group).

Top-16 selection via PACKED fp32 words: each 32-bit word holds the fp16
rounding of negdist2 in its high 16 bits and the 14-bit global support
index in its low 16 bits.  Since every value is <= 0, fp32 ordering of the
packed words equals (negdist2 desc, index asc) - ties break to the lower
index like the reference.  One max8 pass per 2048-column region then gives
top-8 values AND indices at once: no max_index scan, no one-hot resolve.

The packed tile's low (index) lanes are constant per buffer: buffer A
always holds support columns [0,8192), buffer B [8192,16384), so the iota
lanes are DMA'd once at startup and only the fp16 high lanes are rewritten
(strided scalar-engine copy from PSUM) each tile.  The device ships the
top-16 packed words per query row; the host unpacks index + value and
applies sqrt (O(M*K) postprocessing).
"""

import sys

sys.path.insert(0, '/opt/trn_rl_repo')

import numpy as np

B, M, N, C, K = 4, 4096, 16384, 3, 16
NCORES = 8
MPC = M * B // NCORES          # 2048 query rows per core
NT = MPC // 128                # 16 tiles of 128 rows
HALF = N // 2                  # 8192 support cols per packed buffer
REG = 2048                     # max8 region size
NREG_H = HALF // REG           # 4 regions per half

_cache = {}


def _build():
    import concourse.bacc as bacc
    import concourse.mybir as mybir
    import concourse.tile as tile

    dt = mybir.dt
    nc = bacc.Bacc('TRN2', target_bir_lowering=False, debug=False,
                   num_devices=NCORES)
    qaug_d = nc.dram_tensor('qaug', [5, MPC], dt.float32, kind='ExternalInput')
    saug_d = nc.dram_tensor('saug', [5, N], dt.float32, kind='ExternalInput')
    pkinit_d = [nc.dram_tensor(f'pkinit{i}', [128, HALF], dt.float32,
                               kind='ExternalInput') for i in range(2)]
    o_pk = nc.dram_tensor('o_pk', [MPC, K], dt.float32, kind='ExternalOutput')

    with tile.TileContext(nc) as tc:
        with (
            tc.tile_pool(name='big', bufs=1) as big,
            tc.tile_pool(name='med', bufs=2) as med,
            tc.tile_pool(name='small', bufs=4) as small,
            tc.tile_pool(name='ps', bufs=2, space='PSUM') as ps,
        ):
            # one query tile per PE row-group so the 4 replica DMA writes
            # parallelize (same-tile writes serialize in the dep tracker);
            # same for the 8 support column-tiles
            qa = [big.tile([128, MPC], dt.float32, tag=f'qa{g}',
                           name=f'qa{g}') for g in range(4)]
            sa = [big.tile([128, 2048], dt.float32, tag=f'sa{i}',
                           name=f'sa{i}') for i in range(8)]
            # four packed tiles of 4096 cols: (half, quarter) - finer tiles
            # give the scheduler finer hazard granularity
            pk = [[big.tile([128, HALF // 2], dt.float32, tag=f'pk{i}{q}',
                            name=f'pk{i}{q}') for q in range(2)]
                  for i in range(2)]
            for g in range(4):
                nc.sync.dma_start(qa[g][32 * g:32 * g + 5, :], qaug_d[:, :])
            for i in range(8):
                for g in range(4):
                    nc.sync.dma_start(
                        sa[i][32 * g:32 * g + 5, :],
                        saug_d[:, 2048 * i:2048 * (i + 1)])
            # one-time: index iota into the packed buffers' low lanes (the
            # high lanes get overwritten per tile; low lanes never change).
            # 1MB chunks spread over the three DMA-capable queues, pk0 first.
            assign = [nc.gpsimd, nc.scalar, nc.gpsimd, nc.gpsimd,
                      nc.gpsimd, nc.gpsimd, nc.sync, nc.sync]
            n = 0
            for i in range(2):
                for q in range(2):
                    for e in range(2):
                        assign[n].dma_start(
                            pk[i][q][:, 2048 * e:2048 * (e + 1)],
                            pkinit_d[i][:, HALF // 2 * q + 2048 * e:
                                         HALF // 2 * q + 2048 * (e + 1)])
                        n += 1
            for t in range(NT):
                cand = med.tile([128, 64], dt.float32, tag='cand')
                for h in range(2):
                    pk16 = [pk[h][q].bitcast(dt.float16)[:, :].rearrange(
                        'p (w u) -> p w u', u=2) for q in range(2)]
                    for c in range(4):
                        pt = ps.tile([128, 2048], dt.float32, tag='p')
                        for j in range(4):
                            nc.tensor.matmul(
                                pt[:, 512 * j:512 * (j + 1)],
                                qa[j][32 * j:32 * j + 5,
                                      128 * t:128 * (t + 1)],
                                sa[4 * h + c][32 * j:32 * j + 5,
                                              512 * j:512 * (j + 1)],
                                tile_position=(32 * j, 0),
                            )
                        off = 2048 * (c % 2)
                        nc.scalar.activation(
                            pk16[c // 2][:, off:off + 2048, 1], pt[:, :],
                            mybir.ActivationFunctionType.Copy)
                    for r in range(NREG_H):
                        off = REG * (r % 2)
                        nc.vector.max(
                            cand[:, 32 * h + 8 * r:32 * h + 8 * r + 8],
                            pk[h][r // 2][:, off:off + REG])
                # top-16 of the 64 packed candidates (values unique by idx)
                t16 = small.tile([128, K], dt.float32, tag='t16')
                nc.vector.max(t16[:, 0:8], cand[:, :])
                nc.vector.match_replace(cand[:, :], t16[:, 0:8], cand[:, :],
                                        -3.0e38)
                nc.vector.max(t16[:, 8:16], cand[:, :])
                nc.sync.dma_start(o_pk[128 * t:128 * (t + 1), :], t16[:, :])
    nc.compile()
    return nc


def _get_nc():
    if 'nc' not in _cache:
        _cache['nc'] = _build()
    return _cache['nc']


def kernel(query, support, _trace=False):
    from concourse.bass_utils import run_bass_kernel_spmd

    query = np.asarray(query, dtype=np.float32)
    support = np.asarray(support, dtype=np.float32)

    iota = np.arange(N, dtype=np.uint32).view(np.float32)
    pkinit = [np.ascontiguousarray(
        np.broadcast_to(iota[HALF * i:HALF * (i + 1)], (128, HALF)))
        for i in range(2)]
    in_maps = []
    for core in range(NCORES):
        b, h = core // 2, core % 2
        q = query[b, MPC * h:MPC * (h + 1)]          # [2048, 3]
        s = support[b]                                # [16384, 3]
        q2 = (q * q).sum(1)
        s2 = (s * s).sum(1)
        qaug = np.stack([-q2, -np.ones(MPC, np.float32),
                         2 * q[:, 0], 2 * q[:, 1], 2 * q[:, 2]]).astype(np.float32)
        saug = np.stack([np.ones(N, np.float32), s2,
                         s[:, 0], s[:, 1], s[:, 2]]).astype(np.float32)
        in_maps.append({'qaug': qaug, 'saug': saug,
                        'pkinit0': pkinit[0], 'pkinit1': pkinit[1]})

    nc = _get_nc()
    res = run_bass_kernel_spmd(nc, in_maps, list(range(NCORES)), trace=_trace)
    pkw = np.stack([res.results[c]['o_pk'] for c in range(NCORES)])
    pkw = pkw.reshape(B, M, K).view(np.uint32)
    idx = (pkw & 0xFFFF).astype(np.int32)
    v16 = (pkw >> 16).astype(np.uint16).view(np.float16)
    vals = np.sqrt(np.maximum(-v16.astype(np.float32), 0.0))
    if _trace:
        _cache['last_exec_time_ns'] = res.exec_time_ns
    return vals, idx
